# revision 16
# baseline (speedup 1.0000x reference)
"""BigBird block-sparse attention TRN2 kernel (8 NeuronCores, SPMD), v6.

Sharding: core c handles batch b=c//2 and head-half hh=c%2 (8 of 16 heads,
feature slice hh*512..+512).

v6 design: all-bf16, q/k/v SBUF-resident, and phase 2 *overlapped with
phase 1*: seq chunks are processed in order [6, 7, 0, 1..5] so the global
key/value blocks (63 and 0) are ready early, and each middle strip s is
emitted (for all 8 heads) at the position where its two seq chunks are
done. The tensor engine then always has a deep mix of projection matmuls
and attention matmuls -> no idle, HAM stays at full clock, and the
ACT-bound exp stream hides under projection work.

v is computed directly in the shifted "vau" layout vres2[128, 32, 520]:
chunk 0 = [last key block; block 0] (the global PV operand), chunks 1..31
= v rows 64+128j (the sliding-window PV operands), with a ones column per
head group (65-wide groups) for the softmax denominator. Window chunks
that straddle seq-chunk boundaries are computed as two half-tiles (psum
partition halves), so each half depends on a single x chunk. The host
reconstructs plain v from vres2 for the edge-block PV.

Middle strips: scores transposed (keys on partitions), 4 matmuls
[64d,128k].T @ [64d,256q], exp on ACT (psum -> bf16), window mask via one
DVE multiply, P@V against vres2 slices with denominator row. Edge blocks
(0,1,62,63) ship raw bf16 scores (pe1/pe2) per 512-col chunk; host does
exp/mask/softmax/PV.
"""
import sys

if "/opt/trn_rl_repo" not in sys.path:
    sys.path.insert(0, "/opt/trn_rl_repo")

import numpy as np
import ml_dtypes

import concourse.bacc as bacc
import concourse.bass as bass
import concourse.tile as tile
from concourse import mybir
from concourse.bass_utils import run_bass_kernel_spmd

F32 = mybir.dt.float32
BF16 = mybir.dt.bfloat16
NPBF16 = ml_dtypes.bfloat16

B, S, H, HS, D, BLK = 4, 4096, 16, 1024, 64, 64
NB = S // BLK            # 64 key/query blocks
HPC = 8                  # heads per core
FPC = HPC * D            # 512 features per core
NKC = HS // 128          # 8 contraction chunks in phase 1
NSEQ = 8                 # phase-1 seq chunks of 512
NMID = 15                # middle strips of 4 blocks (blocks 2..61)
DUP = 3 * BLK            # duplicated key cols (blocks 0,1,2) at 4096..4288
KTW = S + DUP
VW = HPC * 65            # vres2 row width (64 d + ones per head)

CHUNK_ORDER = [6, 7, 0, 1, 2, 3, 4, 5]
_POS = {c: i for i, c in enumerate(CHUNK_ORDER)}


def _strip_unlocks():
    """position -> list of strip indices that become runnable there."""
    table = {i: [] for i in range(NSEQ)}
    for s in range(NMID):
        c1 = (256 * s + 64) // 512
        c2 = (256 * s + 447) // 512
        p = max(_POS[c1], _POS[c2], _POS[7], _POS[0])
        table[p].append(s)
    return table


_UNLOCK = _strip_unlocks()
# e1 chunk -> emission position (chunk ready, qe ready at pos 2; spread load)
_E1POS = {0: 2, 6: 3, 1: 3, 7: 4, 2: 4, 3: 5, 4: 6, 5: 7}

_BUILT = None


def _build():
    nc = bacc.Bacc(None, target_bir_lowering=False)

    # ---- parameters (bf16 unless noted) ----
    # x8[p, n, c, s'] = X[n*512+s', c*128+p]
    x8 = nc.declare_dram_parameter("x8", [128, NSEQ, NKC, 512], BF16, False)
    # w*[p, c, f] = W.T[c*128+p, f] (wq pre-scaled by 1/8)
    wq = nc.declare_dram_parameter("wq", [128, NKC, FPC], BF16, False)
    wk = nc.declare_dram_parameter("wk", [128, NKC, FPC], BF16, False)
    wv = nc.declare_dram_parameter("wv", [128, NKC, FPC], BF16, False)
    bq = nc.declare_dram_parameter("bq", [128, 4], F32, False)   # pre-scaled
    bk = nc.declare_dram_parameter("bk", [128, 4], F32, False)
    bv = nc.declare_dram_parameter("bv", [FPC], F32, False)
    maskp = nc.declare_dram_parameter("maskp", [128, 4, 256], BF16, False)

    ctxt = nc.declare_dram_parameter("ctxt", [HPC * 65, NMID * 256], BF16, True)
    pe1 = nc.declare_dram_parameter("pe1", [HPC * 128, S], BF16, True)
    pe2 = nc.declare_dram_parameter("pe2", [HPC * 128, 6 * BLK], BF16, True)
    vout2 = nc.declare_dram_parameter("vout2", [128, 32, VW], BF16, True)

    with tile.TileContext(nc) as tc:
        with tc.tile_pool(name="res", bufs=1) as rp, \
             tc.tile_pool(name="xp", bufs=2) as xp, \
             tc.tile_pool(name="bnc", bufs=4) as bnc, \
             tc.tile_pool(name="p2s", bufs=1) as sp, \
             tc.tile_pool(name="ptp", bufs=4) as ptp, \
             tc.tile_pool(name="ps1", bufs=2, space="PSUM") as pp1, \
             tc.tile_pool(name="psqk", bufs=2, space="PSUM") as ppqk, \
             tc.tile_pool(name="pspv", bufs=2, space="PSUM") as pppv:
            kt = rp.tile([128, 4, KTW], BF16, tag="kt", name="kt")
            qt = rp.tile([128, 4, S], BF16, tag="qt", name="qt")
            vres2 = rp.tile([128, 32, VW], BF16, tag="vres2", name="vres2")
            # ones columns (denominator) per 65-wide head group
            v4d = vres2[:].rearrange("p c (h e) -> p c h e", e=65)
            nc.vector.memset(v4d[:, :, :, 64:65], 1.0)
            # first x chunk before the weights (longest pole), split in two
            n0 = CHUNK_ORDER[0]
            xt0 = xp.tile([128, NKC, 512], BF16, tag="xt", name="xt0")
            nc.sync.dma_start(out=xt0[:, 0:4, :], in_=x8[:, n0, 0:4, :])
            nc.sync.dma_start(out=xt0[:, 4:8, :], in_=x8[:, n0, 4:8, :])
            wts = {}
            for name, w in (("v", wv), ("k", wk), ("q", wq)):
                t = rp.tile([128, NKC, FPC], BF16, tag=f"w{name}", name=f"w{name}")
                nc.scalar.dma_start(out=t[:], in_=w[:])
                wts[name] = t
            bqt = rp.tile([128, 4], F32, tag="bqt", name="bqt")
            bkt = rp.tile([128, 4], F32, tag="bkt", name="bkt")
            nc.gpsimd.dma_start(out=bqt[:], in_=bq[:])
            nc.gpsimd.dma_start(out=bkt[:], in_=bk[:])
            bvt = rp.tile([128, FPC], F32, tag="bvt", name="bvt")
            bv_ap = bv.ap()
            nc.gpsimd.dma_start(
                out=bvt[:],
                in_=bass.AP(tensor=bv_ap.tensor, offset=bv_ap.offset,
                            ap=[[0, 128]] + bv_ap.ap),
            )
            maskt = rp.tile([128, 4, 256], BF16, tag="maskt", name="maskt")
            nc.gpsimd.dma_start(out=maskt[:], in_=maskp[:])

            st = dict(nc=nc, x8=x8, xp=xp, pp1=pp1, ppqk=ppqk, pppv=pppv,
                      bnc=bnc, wts=wts, bqt=bqt, bkt=bkt, bvt=bvt, kt=kt,
                      qt=qt, vres2=vres2, maskt=maskt, sp=sp, ptp=ptp,
                      ctxt=ctxt, pe1=pe1, pe2=pe2, vout2=vout2, xt0=xt0,
                      qes={})
            for p, n in enumerate(CHUNK_ORDER):
                _chunk(st, p, n)
            # ship vres2 for the host edge PV
            nc.gpsimd.dma_start(out=vout2[:], in_=vres2[:])
    nc.compile()
    return nc


def _vadd(st, dst, ps, lo, hi):
    """psum [lo:hi, 512] + bias -> vres2 rows lo:hi of chunk view dst
    (65-wide head groups, skipping the ones column)."""
    nc, bvt = st["nc"], st["bvt"]
    d3 = dst.rearrange("p (h e) -> p h e", e=65)[lo:hi, :, 0:64]
    p3 = ps[lo:hi, :].rearrange("p (h d) -> p h d", d=64)
    b3 = bvt[lo:hi, :].rearrange("p (h d) -> p h d", d=64)
    nc.vector.tensor_add(d3, p3, b3)


def _vtile_full(st, xtile, c0, cj):
    """one within-chunk shifted v tile: x cols c0:c0+128 -> vres2 chunk cj."""
    nc, pp1, wts = st["nc"], st["pp1"], st["wts"]
    ps = pp1.tile([128, 512], F32, tag="ps", name="psv")
    for kc in range(NKC):
        nc.tensor.matmul(ps[:], xtile[:, kc, c0:c0 + 128], wts["v"][:, kc, :],
                         start=(kc == 0), stop=(kc == NKC - 1))
    _vadd(st, st["vres2"][:, cj, :], ps, 0, 128)


def _vtile_half(st, xtile, c0, cj, half):
    """half (64 seq rows) of a boundary-straddling shifted v tile.
    half=0 -> psum/vres2 partitions 0:64, half=1 -> 64:128."""
    nc, pp1, wts = st["nc"], st["pp1"], st["wts"]
    lo, hi = half * 64, half * 64 + 64
    ps = pp1.tile([128, 512], F32, tag="ps", name="psvh")
    for kc in range(NKC):
        nc.tensor.matmul(ps[lo:hi, :], xtile[:, kc, c0:c0 + 64],
                         wts["v"][:, kc, :],
                         start=(kc == 0), stop=(kc == NKC - 1))
    _vadd(st, st["vres2"][:, cj, :], ps, lo, hi)


def _chunk(st, p, n):
    """everything emitted at position p (seq chunk n): projection work,
    newly-unlocked strips (all heads), edge-score chunks."""
    nc, x8, xp, pp1 = st["nc"], st["x8"], st["xp"], st["pp1"]
    wts, kt, qt = st["wts"], st["kt"], st["qt"]
    if p == 0:
        xtile = st["xt0"]
    else:
        xtile = xp.tile([128, NKC, 512], BF16, tag="xt", name=f"xt{n}")
        nc.sync.dma_start(out=xtile[:], in_=x8[:, n])
    # ---- v: 3 within tiles + up-to-2 boundary halves ----
    for i in range(3):
        _vtile_full(st, xtile, 64 + 128 * i, 4 * n + 1 + i)
    if n < 7:
        _vtile_half(st, xtile, 448, 4 * n + 4, 0)   # rows 512n+448..512
    else:
        _vtile_half(st, xtile, 448, 0, 0)           # global: last 64 rows
    if n > 0:
        _vtile_half(st, xtile, 0, 4 * n, 1)         # rows 512n..512n+64
    else:
        _vtile_half(st, xtile, 0, 0, 1)             # global: rows 0..64
    # ---- k, q m-tiles ----
    for name, dst, bt in (("k", kt, st["bkt"]), ("q", qt, st["bqt"])):
        for m in range(4):
            ps = pp1.tile([128, 512], F32, tag="ps", name="ps")
            for kc in range(NKC):
                nc.tensor.matmul(
                    ps[:], wts[name][:, kc, m * 128:(m + 1) * 128],
                    xtile[:, kc, :], start=(kc == 0), stop=(kc == NKC - 1))
            nc.scalar.activation(
                dst[:, m, n * 512:(n + 1) * 512], ps[:],
                mybir.ActivationFunctionType.Identity,
                bias=bt[:, m:m + 1], scale=1.0)
            if n == 0 and name == "k":
                nc.vector.tensor_copy(kt[:, m, S:KTW], kt[:, m, 0:DUP])
    # ---- edge queries + e2 once both chunks 7 and 0 are in (pos 2) ----
    if p == 2:
        for h in range(HPC):
            m, hp = h // 2, (h % 2) * 64
            qe = st["sp"].tile([128, 256], BF16, tag=f"qe{h}", name=f"qe{h}")
            for i, qc in enumerate((0, 63, 1, 62)):
                nc.vector.tensor_copy(qe[hp:hp + 64, i * 64:(i + 1) * 64],
                                      qt[hp:hp + 64, m, qc * BLK:(qc + 1) * BLK])
            st["qes"][h] = qe
        for h in range(HPC):
            _e2(st, h)
    # ---- newly-unlocked strips, all heads ----
    for s in _UNLOCK[p]:
        for h in range(HPC):
            _p2strip(st, h, s)
    # ---- edge-score chunks assigned to this position ----
    for c, cpos in _E1POS.items():
        if cpos == p:
            for h in range(HPC):
                _e1chunk(st, h, c)


def _p2strip(st, h, s):
    nc, kt, qt, vres2 = st["nc"], st["kt"], st["qt"], st["vres2"]
    m, hp = h // 2, (h % 2) * 64
    w = 4 * s + 2
    q0 = w * BLK
    sps = st["ppqk"].tile([128, 4, 256], F32, tag="qk", name="qk")
    rhs = qt[hp:hp + 64, m, q0:q0 + 256]
    # chunk 0: global keys = block 63 | dup block 0 (cols 4032:4160)
    nc.tensor.matmul(sps[:, 0, :], kt[hp:hp + 64, m, S - BLK:S + BLK],
                     rhs, start=True, stop=True)
    for c in range(3):
        col = (w - 1 + 2 * c) * BLK
        nc.tensor.matmul(sps[:, 1 + c, :], kt[hp:hp + 64, m, col:col + 128],
                         rhs, start=True, stop=True)
    pt = st["ptp"].tile([128, 4, 256], BF16, tag="pt", name="pt")
    nc.scalar.activation(pt[:], sps[:], mybir.ActivationFunctionType.Exp)
    nc.vector.tensor_mul(pt[:], pt[:], st["maskt"][:])
    cps = st["pppv"].tile([65, 256], F32, tag="pv", name="pv")
    g = slice(h * 65, h * 65 + 65)
    nc.tensor.matmul(cps[:], vres2[:, 0, g], pt[:, 0, :], start=True, stop=False)
    for c in range(3):
        nc.tensor.matmul(cps[:], vres2[:, 1 + 2 * s + c, g], pt[:, 1 + c, :],
                         start=False, stop=(c == 2))
    cb = st["bnc"].tile([65, 256], BF16, tag="cb", name="cb")
    nc.vector.tensor_copy(cb[:], cps[:])
    nc.gpsimd.dma_start(
        out=st["ctxt"][h * 65:(h + 1) * 65, s * 256:(s + 1) * 256], in_=cb[:])


def _e1chunk(st, h, c):
    nc, kt, pp1 = st["nc"], st["kt"], st["pp1"]
    m, hp = h // 2, (h % 2) * 64
    qe = st["qes"][h]
    eps = pp1.tile([128, 512], F32, tag="ps", name="eps")
    nc.tensor.matmul(eps[:], qe[hp:hp + 64, 0:128],
                     kt[hp:hp + 64, m, c * 512:(c + 1) * 512],
                     start=True, stop=True)
    eb = st["bnc"].tile([128, 512], BF16, tag="eb", name="eb")
    nc.vector.tensor_copy(eb[:], eps[:])
    nc.gpsimd.dma_start(
        out=st["pe1"][h * 128:(h + 1) * 128, c * 512:(c + 1) * 512], in_=eb[:])


def _e2(st, h):
    nc, kt, pp1 = st["nc"], st["kt"], st["pp1"]
    m, hp = h // 2, (h % 2) * 64
    qe = st["qes"][h]
    e2ps = pp1.tile([128, 512], F32, tag="ps", name="e2ps")
    nc.tensor.matmul(e2ps[:, 0:384], qe[hp:hp + 64, 128:256],
                     kt[hp:hp + 64, m, S - 3 * BLK:KTW],
                     start=True, stop=True)
    eb = st["bnc"].tile([128, 512], BF16, tag="eb", name="e2b")
    nc.vector.tensor_copy(eb[:, 0:384], e2ps[:, 0:384])
    nc.gpsimd.dma_start(out=st["pe2"][h * 128:(h + 1) * 128, :],
                        in_=eb[:, 0:384])


def _wshuf(W, fs, scl=1.0):
    # [p, c, f]: W.T[c*128+p, f] for the feature slice
    wt = np.asarray(W, np.float32)[fs, :].T * np.float32(scl)  # [HS, FPC]
    return np.ascontiguousarray(
        wt.reshape(NKC, 128, FPC).transpose(1, 0, 2)).astype(NPBF16)


def _mask_tile():
    # mask[k(2 blocks x 64), c, q(4 blocks x 64)]; c=0 global -> all ones.
    mask = np.zeros((128, 4, 256), np.float32)
    mask[:, 0, :] = 1.0
    for c in range(1, 4):
        for kb in range(2):
            krel = -1 + 2 * (c - 1) + kb
            for j in range(4):
                if j - 1 <= krel <= j + 1:
                    mask[kb * 64:(kb + 1) * 64, c, j * 64:(j + 1) * 64] = 1.0
    return mask.astype(NPBF16)


_MASK = _mask_tile()


def _host_inputs(hidden, Wq, bq, Wk, bk, Wv, bv, c):
    b, hh = c // 2, c % 2
    fs = slice(hh * FPC, (hh + 1) * FPC)
    X = np.asarray(hidden[b], np.float32)
    x8 = np.ascontiguousarray(
        X.reshape(NSEQ, 512, NKC, 128).transpose(3, 0, 2, 1)).astype(NPBF16)
    scl = np.float32(0.125)
    return {
        "x8": x8,
        "wq": _wshuf(Wq, fs, 0.125),
        "wk": _wshuf(Wk, fs),
        "wv": _wshuf(Wv, fs),
        "bq": (np.asarray(bq)[fs] * scl).astype(np.float32).reshape(4, 128).T.copy(),
        "bk": np.asarray(bk)[fs].astype(np.float32).reshape(4, 128).T.copy(),
        "bv": np.asarray(bv)[fs].astype(np.float32),
        "maskp": _MASK,
    }


def _host_finish(res_c):
    """Per-core host post-processing -> [S, FPC] output slice."""
    ctxt = np.asarray(res_c["ctxt"], np.float32).reshape(HPC, 65, NMID * 256)
    p1 = np.asarray(res_c["pe1"], np.float32).reshape(HPC, 128, S)
    p2 = np.asarray(res_c["pe2"], np.float32).reshape(HPC, 128, 6 * BLK)
    # reconstruct v [S, FPC] from the shifted vres2 layout
    v2 = np.asarray(res_c["vout2"], np.float32)          # [128, 32, 520]
    hd = v2.reshape(128, 32, HPC, 65)[:, :, :, 0:64]     # [p, c, h, d]
    v = np.empty((S, FPC), np.float32)
    v[64:4032] = hd[:, 1:32].transpose(1, 0, 2, 3).reshape(31 * 128, FPC)
    v[4032:4096] = hd[0:64, 0].reshape(64, FPC)
    v[0:64] = hd[64:128, 0].reshape(64, FPC)
    out = np.empty((S, FPC), np.float32)
    for h in range(HPC):
        cs = slice(h * 64, (h + 1) * 64)
        vh = v[:, cs]
        num = ctxt[h, 0:64, :]
        den = ctxt[h, 64, :]
        out[2 * BLK:62 * BLK, cs] = (num / den).T
        P = np.exp(p1[h])
        C = (P / P.sum(1, keepdims=True)) @ vh
        out[0:BLK, cs] = C[0:64]
        out[S - BLK:S, cs] = C[64:128]
        P = np.exp(p2[h])
        P[0:64, 0:128] = 0.0      # block 1 bans blocks 61, 62
        P[64:128, 256:384] = 0.0  # block 62 bans blocks 1, 2
        vk = np.concatenate([vh[(NB - 3) * BLK:], vh[0:3 * BLK]], 0)
        C = (P / P.sum(1, keepdims=True)) @ vk
        out[BLK:2 * BLK, cs] = C[0:64]
        out[62 * BLK:63 * BLK, cs] = C[64:128]
    return out


def _run(inputs, trace=False):
    global _BUILT
    if _BUILT is None:
        _BUILT = _build()
    core_ids = list(range(8))
    in_maps = [_host_inputs(**inputs, c=c) for c in core_ids]
    res = run_bass_kernel_spmd(_BUILT, in_maps, core_ids, trace=trace)
    out = np.empty((B, S, HS), np.float32)
    for c in core_ids:
        b, hh = c // 2, c % 2
        out[b, :, hh * FPC:(hh + 1) * FPC] = _host_finish(res.results[c])
    return out, res


def kernel(hidden_states, Wq, bq, Wk, bk, Wv, bv):
    inputs = dict(hidden=np.asarray(hidden_states), Wq=np.asarray(Wq),
                  bq=np.asarray(bq), Wk=np.asarray(Wk),
                  bk=np.asarray(bk), Wv=np.asarray(Wv), bv=np.asarray(bv))
    out, _ = _run(inputs, trace=False)
    return out


# revision 19
# speedup vs baseline: 1.0873x; 1.0873x over previous
"""BigBird block-sparse attention TRN2 kernel (8 NeuronCores, SPMD), v3.

Sharding: core c handles batch b=c//2 and head-half hh=c%2 (8 of 16 heads,
feature slice hh*512..+512).

v3 design: all-bf16 dataflow, q/k/v SBUF-resident (no DRAM round trip),
attention matmuls read the projection m-tiles directly via partition
slices, window masking via one DVE multiply against a constant mask tile.
Phase 1 is split into m-passes (x re-loaded per pass) and interleaved
with per-head-pair phase 2 so the tensor engine never starves (keeps the
HAM clock gate at full speed):

  mpass0: v (all chunks) + k/q m-tile 0      -> heads 0,1 ready
  pair01 | mpass1 -> pair23 | mpass2 -> pair45 | mpass3 -> pair67

Phase 2 per head h (m = h//2, partitions (h%2)*64..+64): middle query
blocks 2..61 in 15 strips of 4 blocks. Scores computed transposed (keys
on partitions): 4 matmuls [64d,128k].T @ [64d,256q] per strip (chunk 0 =
global keys 63|0 via duplicated block-0 cols, chunks 1-3 sliding window).
exp on ACT (PSUM -> SBUF bf16), window mask via DVE mult, P@V with vau
tiles (keys on partitions, ones column for the denominator). num/den ship
bf16; host normalizes + transposes. Edge blocks (0,1,62,63) ship raw bf16
scores; host does exp/mask/softmax/PV against bf16 v (vout).
"""
import sys

if "/opt/trn_rl_repo" not in sys.path:
    sys.path.insert(0, "/opt/trn_rl_repo")

import numpy as np
import ml_dtypes

import concourse.bacc as bacc
import concourse.bass as bass
import concourse.tile as tile
from concourse import mybir
from concourse.bass_utils import run_bass_kernel_spmd

F32 = mybir.dt.float32
BF16 = mybir.dt.bfloat16
NPBF16 = ml_dtypes.bfloat16

B, S, H, HS, D, BLK = 4, 4096, 16, 1024, 64, 64
NB = S // BLK            # 64 key/query blocks
HPC = 8                  # heads per core
FPC = HPC * D            # 512 features per core
NKC = HS // 128          # 8 contraction chunks in phase 1
NSEQ = 8                 # phase-1 seq chunks of 512
NMID = 15                # middle strips of 4 blocks (blocks 2..61)
DUP = 3 * BLK            # duplicated key cols (blocks 0,1,2) at 4096..4288
KTW = S + DUP

_BUILT = None


def _build():
    nc = bacc.Bacc(None, target_bir_lowering=False)

    # ---- parameters (bf16 unless noted) ----
    # x8[p, n, c, s'] = X[n*512+s', c*128+p]
    x8 = nc.declare_dram_parameter("x8", [128, NSEQ, NKC, 512], BF16, False)
    # w*[p, c, f] = W.T[c*128+p, f] (wq pre-scaled by 1/8)
    wq = nc.declare_dram_parameter("wq", [128, NKC, FPC], BF16, False)
    wk = nc.declare_dram_parameter("wk", [128, NKC, FPC], BF16, False)
    wv = nc.declare_dram_parameter("wv", [128, NKC, FPC], BF16, False)
    bq = nc.declare_dram_parameter("bq", [128, 4], F32, False)   # pre-scaled
    bk = nc.declare_dram_parameter("bk", [128, 4], F32, False)
    bv = nc.declare_dram_parameter("bv", [FPC], F32, False)
    maskp = nc.declare_dram_parameter("maskp", [128, 4, 256], BF16, False)
    onesp = nc.declare_dram_parameter("onesp", [128, 32, 1], BF16, False)

    ctxt = nc.declare_dram_parameter("ctxt", [HPC * 65, NMID * 256], BF16, True)
    pe1 = nc.declare_dram_parameter("pe1", [HPC * 128, S], BF16, True)
    pe2 = nc.declare_dram_parameter("pe2", [HPC * 128, 6 * BLK], BF16, True)
    vout = nc.declare_dram_parameter("vout", [S, FPC], BF16, True)

    with tile.TileContext(nc) as tc:
        with tc.tile_pool(name="res", bufs=1) as rp, \
             tc.tile_pool(name="xp", bufs=2) as xp, \
             tc.tile_pool(name="vaup", bufs=1) as vp, \
             tc.tile_pool(name="p2s", bufs=1) as sp, \
             tc.tile_pool(name="ptp", bufs=4) as ptp, \
             tc.tile_pool(name="ps1", bufs=2, space="PSUM") as pp1, \
             tc.tile_pool(name="psqk", bufs=2, space="PSUM") as ppqk, \
             tc.tile_pool(name="pspv", bufs=2, space="PSUM") as pppv:
            # resident tiles
            kt = rp.tile([128, 4, KTW], BF16, tag="kt", name="kt")
            qt = rp.tile([128, 4, S], BF16, tag="qt", name="qt")
            vres = rp.tile([128, 32, FPC], BF16, tag="vres", name="vres")
            # first x chunk before the weights: it is the longest pole
            # (split in two so the first matmuls can start sooner)
            xt0 = xp.tile([128, NKC, 512], BF16, tag="xt", name="xt0")
            nc.sync.dma_start(out=xt0[:, :, 0:256], in_=x8[:, 0, :, 0:256])
            nc.sync.dma_start(out=xt0[:, :, 256:512], in_=x8[:, 0, :, 256:512])
            wts = {}
            for name, w in (("v", wv), ("k", wk), ("q", wq)):
                t = rp.tile([128, NKC, FPC], BF16, tag=f"w{name}", name=f"w{name}")
                nc.scalar.dma_start(out=t[:], in_=w[:])
                wts[name] = t
            bqt = rp.tile([128, 4], F32, tag="bqt", name="bqt")
            bkt = rp.tile([128, 4], F32, tag="bkt", name="bkt")
            nc.gpsimd.dma_start(out=bqt[:], in_=bq[:])
            nc.gpsimd.dma_start(out=bkt[:], in_=bk[:])
            bvt = rp.tile([128, FPC], F32, tag="bvt", name="bvt")
            bv_ap = bv.ap()
            nc.gpsimd.dma_start(
                out=bvt[:],
                in_=bass.AP(tensor=bv_ap.tensor, offset=bv_ap.offset,
                            ap=[[0, 128]] + bv_ap.ap),
            )
            maskt = rp.tile([128, 4, 256], BF16, tag="maskt", name="maskt")
            nc.gpsimd.dma_start(out=maskt[:], in_=maskp[:])
            vaus = []
            for vi in range(2):
                vt = vp.tile([128, 32, 65], BF16, tag=f"vau{vi}", name=f"vau{vi}")
                nc.gpsimd.dma_start(out=vt[:, :, 64:65], in_=onesp[:])
                vaus.append(vt)

            st = dict(nc=nc, x8=x8, xp=xp, pp1=pp1, ppqk=ppqk, pppv=pppv,
                      wts=wts, bqt=bqt, bkt=bkt, bvt=bvt, kt=kt, qt=qt,
                      vres=vres, vout=vout, vaus=vaus, maskt=maskt, sp=sp,
                      ptp=ptp, ctxt=ctxt, pe1=pe1, pe2=pe2, xt0=xt0)
            _phase1(st)
            hs_cur = _p2loads(st, 0)
            for h in range(HPC):
                hs_next = _p2head(st, h, hs_cur)
                hs_cur = hs_next
    nc.compile()
    return nc


def _phase1(st):
    """QKV projection over seq chunks: per chunk v first (so vau loads can
    start at the earliest), then k, then q."""
    nc, x8, xp, pp1 = st["nc"], st["x8"], st["xp"], st["pp1"]
    wts, kt, qt, vres, vout = st["wts"], st["kt"], st["qt"], st["vres"], st["vout"]
    for n in range(NSEQ):
        if n == 0:
            xtile = st["xt0"]
        else:
            xtile = xp.tile([128, NKC, 512], BF16, tag="xt", name=f"xt{n}")
            nc.sync.dma_start(out=xtile[:], in_=x8[:, n])
        for sm in range(4):
            ps = pp1.tile([128, 512], F32, tag="ps", name="psv")
            for kc in range(NKC):
                nc.tensor.matmul(
                    ps[:],
                    xtile[:, kc, sm * 128:(sm + 1) * 128],
                    wts["v"][:, kc, :],
                    start=(kc == 0), stop=(kc == NKC - 1),
                )
            ch = 4 * n + sm
            nc.vector.tensor_add(vres[:, ch, :], ps[:], st["bvt"][:])
            nc.gpsimd.dma_start(
                out=vout[n * 512 + sm * 128: n * 512 + (sm + 1) * 128, :],
                in_=vres[:, ch, :],
            )
        for name, dst, bt in (("k", kt, st["bkt"]), ("q", qt, st["bqt"])):
            for m in range(4):
                ps = pp1.tile([128, 512], F32, tag="ps", name="ps")
                for kc in range(NKC):
                    nc.tensor.matmul(
                        ps[:],
                        wts[name][:, kc, m * 128:(m + 1) * 128],
                        xtile[:, kc, :],
                        start=(kc == 0), stop=(kc == NKC - 1),
                    )
                nc.scalar.activation(
                    dst[:, m, n * 512:(n + 1) * 512], ps[:],
                    mybir.ActivationFunctionType.Identity,
                    bias=bt[:, m:m + 1], scale=1.0,
                )
                if n == 0 and name == "k":
                    # duplicate key blocks 0,1,2 for middle-global + e2
                    nc.vector.tensor_copy(kt[:, m, S:KTW], kt[:, m, 0:DUP])


def _p2loads(st, h):
    nc, vres, qt, sp = st["nc"], st["vres"], st["qt"], st["sp"]
    m, hp = h // 2, (h % 2) * 64
    # vau: keys on partitions; chunk 0 = [last block; block 0],
    # chunks 1..31 = rows 64+128j (shifted window chunks)
    vau = st["vaus"][h % 2]
    nc.gpsimd.dma_start(out=vau[0:64, 0, 0:64],
                        in_=vres[64:128, 31, h * 64:(h + 1) * 64])
    nc.gpsimd.dma_start(out=vau[64:128, 0, 0:64],
                        in_=vres[0:64, 0, h * 64:(h + 1) * 64])
    nc.gpsimd.dma_start(out=vau[0:64, 1:32, 0:64],
                        in_=vres[64:128, 0:31, h * 64:(h + 1) * 64])
    nc.gpsimd.dma_start(out=vau[64:128, 1:32, 0:64],
                        in_=vres[0:64, 1:32, h * 64:(h + 1) * 64])
    # qe: edge query blocks [0, 63, 1, 62] on partitions hp..hp+64
    qe = sp.tile([128, 256], BF16, tag=f"qe{h % 2}", name=f"qe{h % 2}")
    for i, qc in enumerate((0, 63, 1, 62)):
        nc.vector.tensor_copy(qe[hp:hp + 64, i * 64:(i + 1) * 64],
                              qt[hp:hp + 64, m, qc * BLK:(qc + 1) * BLK])
    ctx_acc = sp.tile([65, NMID * 256], BF16, tag=f"ctx{h % 2}",
                      name=f"ctx{h % 2}")
    return vau, qe, ctx_acc


def _p2strip(st, h, s, hs):
    nc, kt, qt = st["nc"], st["kt"], st["qt"]
    m, hp = h // 2, (h % 2) * 64
    vau, qe, ctx_acc = hs
    w = 4 * s + 2
    q0 = w * BLK
    sps = st["ppqk"].tile([128, 4, 256], F32, tag="qk", name="qk")
    rhs = qt[hp:hp + 64, m, q0:q0 + 256]
    # chunk 0: global keys = block 63 | dup block 0 (cols 4032:4160)
    nc.tensor.matmul(sps[:, 0, :], kt[hp:hp + 64, m, S - BLK:S + BLK],
                     rhs, start=True, stop=True)
    for c in range(3):
        col = (w - 1 + 2 * c) * BLK
        nc.tensor.matmul(sps[:, 1 + c, :], kt[hp:hp + 64, m, col:col + 128],
                         rhs, start=True, stop=True)
    pt = st["ptp"].tile([128, 4, 256], BF16, tag="pt", name="pt")
    nc.scalar.activation(pt[:], sps[:], mybir.ActivationFunctionType.Exp)
    # chunk 0 (global) is unmasked; only the 3 window chunks need the mask
    nc.vector.tensor_mul(pt[:, 1:4, :], pt[:, 1:4, :], st["maskt"][:, 1:4, :])
    cps = st["pppv"].tile([65, 256], F32, tag="pv", name="pv")
    nc.tensor.matmul(cps[:], vau[:, 0, :], pt[:, 0, :], start=True, stop=False)
    for c in range(3):
        nc.tensor.matmul(cps[:], vau[:, 1 + 2 * s + c, :], pt[:, 1 + c, :],
                         start=False, stop=(c == 2))
    nc.vector.tensor_copy(ctx_acc[:, s * 256:(s + 1) * 256], cps[:])


def _e1mm(st, h, hs, c):
    nc, kt, pp1 = st["nc"], st["kt"], st["pp1"]
    m, hp = h // 2, (h % 2) * 64
    vau, qe, ctx_acc = hs
    eps = pp1.tile([128, 512], F32, tag="ps", name="eps")
    nc.tensor.matmul(eps[:], qe[hp:hp + 64, 0:128],
                     kt[hp:hp + 64, m, c * 512:(c + 1) * 512],
                     start=True, stop=True)
    return eps


def _e2(st, h, hs):
    nc, kt, sp, pp1 = st["nc"], st["kt"], st["sp"], st["pp1"]
    m, hp = h // 2, (h % 2) * 64
    vau, qe, ctx_acc = hs
    # e2: blocks 1 and 62; keys = blocks 61,62,63 | dup 0,1,2 (cols 3904:4288)
    e2ps = pp1.tile([128, 512], F32, tag="ps", name="e2ps")
    nc.tensor.matmul(e2ps[:, 0:384], qe[hp:hp + 64, 128:256],
                     kt[hp:hp + 64, m, S - 3 * BLK:KTW],
                     start=True, stop=True)
    e2ev = sp.tile([128, 6 * BLK], BF16, tag=f"pe2{h % 2}", name=f"pe2{h % 2}")
    nc.vector.tensor_copy(e2ev[:], e2ps[:, 0:384])
    nc.gpsimd.dma_start(out=st["pe2"][h * 128:(h + 1) * 128, :], in_=e2ev[:])


def _e1chunk(st, h, hs, pev, c):
    nc = st["nc"]
    eps = _e1mm(st, h, hs, c)
    nc.vector.tensor_copy(pev[:, c * 512:(c + 1) * 512], eps[:])


def _p2head(st, h, hs):
    """One head's phase 2 with edge matmuls woven between strips (PE filler
    so the tensor engine never starves on the ACT-bound strip cadence)."""
    nc, sp = st["nc"], st["sp"]
    pev = sp.tile([128, S], BF16, tag=f"pe1{h % 2}", name=f"pe1{h % 2}")
    _e1chunk(st, h, hs, pev, 0)
    _e1chunk(st, h, hs, pev, 1)
    hs_next = None
    for s in range(NMID):
        _p2strip(st, h, s, hs)
        if 1 <= s <= 6:
            _e1chunk(st, h, hs, pev, s + 1)
        elif s == 7:
            _e2(st, h, hs)
        elif s == 8 and h + 1 < HPC:
            hs_next = _p2loads(st, h + 1)
    nc.gpsimd.dma_start(out=st["pe1"][h * 128:(h + 1) * 128, :], in_=pev[:])
    nc.gpsimd.dma_start(out=st["ctxt"][h * 65:(h + 1) * 65, :], in_=hs[2][:])
    return hs_next


def _wshuf(W, fs, scl=1.0):
    # [p, c, f]: W.T[c*128+p, f] for the feature slice
    wt = np.asarray(W, np.float32)[fs, :].T * np.float32(scl)  # [HS, FPC]
    return np.ascontiguousarray(
        wt.reshape(NKC, 128, FPC).transpose(1, 0, 2)).astype(NPBF16)


def _mask_tile():
    # mask[k(2 blocks x 64), c, q(4 blocks x 64)]; c=0 global -> all ones.
    # window chunks c=1..3 hold key blocks (w-1+2(c-1), w+2(c-1)); q-block
    # j=0..3 = w+j attends blocks w+j-1..w+j+1.
    mask = np.zeros((128, 4, 256), np.float32)
    mask[:, 0, :] = 1.0
    for c in range(1, 4):
        for kb in range(2):            # key block (w-1+2(c-1)) + kb
            krel = -1 + 2 * (c - 1) + kb
            for j in range(4):         # q block w+j
                if j - 1 <= krel <= j + 1:
                    mask[kb * 64:(kb + 1) * 64, c, j * 64:(j + 1) * 64] = 1.0
    return mask.astype(NPBF16)


_MASK = _mask_tile()
_ONES = np.ones((128, 32, 1), NPBF16)


def _host_inputs(hidden, Wq, bq, Wk, bk, Wv, bv, c):
    b, hh = c // 2, c % 2
    fs = slice(hh * FPC, (hh + 1) * FPC)
    X = np.asarray(hidden[b], np.float32)
    x8 = np.ascontiguousarray(
        X.reshape(NSEQ, 512, NKC, 128).transpose(3, 0, 2, 1)).astype(NPBF16)
    scl = np.float32(0.125)
    return {
        "x8": x8,
        "wq": _wshuf(Wq, fs, 0.125),
        "wk": _wshuf(Wk, fs),
        "wv": _wshuf(Wv, fs),
        "bq": (np.asarray(bq)[fs] * scl).astype(np.float32).reshape(4, 128).T.copy(),
        "bk": np.asarray(bk)[fs].astype(np.float32).reshape(4, 128).T.copy(),
        "bv": np.asarray(bv)[fs].astype(np.float32),
        "maskp": _MASK,
        "onesp": _ONES,
    }


def _host_finish(res_c):
    """Per-core host post-processing -> [S, FPC] output slice."""
    ctxt = np.asarray(res_c["ctxt"], np.float32).reshape(HPC, 65, NMID * 256)
    p1 = np.asarray(res_c["pe1"], np.float32).reshape(HPC, 128, S)
    p2 = np.asarray(res_c["pe2"], np.float32).reshape(HPC, 128, 6 * BLK)
    v = np.asarray(res_c["vout"], np.float32)  # [S, FPC]
    out = np.empty((S, FPC), np.float32)
    for h in range(HPC):
        cs = slice(h * 64, (h + 1) * 64)
        vh = v[:, cs]
        # middle blocks 2..61
        num = ctxt[h, 0:64, :]
        den = ctxt[h, 64, :]
        out[2 * BLK:62 * BLK, cs] = (num / den).T
        # e1: blocks 0, 63 (full attention); device ships raw bf16 scores
        P = np.exp(p1[h])
        C = (P / P.sum(1, keepdims=True)) @ vh
        out[0:BLK, cs] = C[0:64]
        out[S - BLK:S, cs] = C[64:128]
        # e2: blocks 1, 62; key cols = blocks [61, 62, 63, 0, 1, 2]
        P = np.exp(p2[h])
        P[0:64, 0:128] = 0.0      # block 1 bans blocks 61, 62
        P[64:128, 256:384] = 0.0  # block 62 bans blocks 1, 2
        vk = np.concatenate([vh[(NB - 3) * BLK:], vh[0:3 * BLK]], 0)
        C = (P / P.sum(1, keepdims=True)) @ vk
        out[BLK:2 * BLK, cs] = C[0:64]
        out[62 * BLK:63 * BLK, cs] = C[64:128]
    return out


def _run(inputs, trace=False):
    global _BUILT
    if _BUILT is None:
        _BUILT = _build()
    core_ids = list(range(8))
    in_maps = [_host_inputs(**inputs, c=c) for c in core_ids]
    res = run_bass_kernel_spmd(_BUILT, in_maps, core_ids, trace=trace)
    out = np.empty((B, S, HS), np.float32)
    for c in core_ids:
        b, hh = c // 2, c % 2
        out[b, :, hh * FPC:(hh + 1) * FPC] = _host_finish(res.results[c])
    return out, res


def kernel(hidden_states, Wq, bq, Wk, bk, Wv, bv):
    inputs = dict(hidden=np.asarray(hidden_states), Wq=np.asarray(Wq),
                  bq=np.asarray(bq), Wk=np.asarray(Wk),
                  bk=np.asarray(bk), Wv=np.asarray(Wv), bv=np.asarray(bv))
    out, _ = _run(inputs, trace=False)
    return out
